# revision 18
# baseline (speedup 1.0000x reference)
"""Trainium2 Bass kernel for nn_DIMPA (3-hop dual-graph COO SpMM).

Strategy (8 NeuronCores, SPMD single program):
  - Destination nodes sharded across cores (12500 rows/core).
  - Host pre-sorts each core's edges by (dest-tile-of-128, src-quartile),
    builds int16 gather indices (quartile-relative, so they fit int16),
    f32 edge values and f32 local-dest ids, laid out per 128-edge chunk.
  - Device, per dest tile: SWDGE dma_gather of source rows from HBM,
    DVE builds a one-hot "segment matrix" (iota == dst_local) and scales
    gathered rows by edge values, PE computes onehot.T @ feats which IS
    the segment-sum (scatter-add) into PSUM, accumulated over chunks.
  - feat accumulators (w[h] * curr_h) live in SBUF for the whole kernel.
  - After hops 1 and 2, an AllGather rebuilds the full N x D "curr" in
    each core's HBM to serve as the next hop's gather source.
"""

import math
from contextlib import ExitStack

import numpy as np

import concourse.bass as bass
import concourse.bacc as bacc
import concourse.tile as tile
from concourse import library_config, mybir
from concourse.bass_utils import run_bass_kernel_spmd

F32 = mybir.dt.float32
BF16 = mybir.dt.bfloat16
I16 = mybir.dt.int16
I32 = mybir.dt.int32


class Cfg:
    def __init__(self, N=100000, E=1200000, D=64, HOP=3, CORES=8, NQ=4,
                 debug=False, mm_bf16=False, cnt_reg=False):
        assert N % CORES == 0 and N % NQ == 0
        self.N, self.E, self.D, self.HOP, self.CORES, self.NQ = N, E, D, HOP, CORES, NQ
        self.NPC = N // CORES              # nodes per core
        self.TILES = math.ceil(self.NPC / 128)
        self.TAIL = self.NPC - (self.TILES - 1) * 128
        self.QROWS = N // NQ               # rows per source quartile
        assert self.QROWS <= 32767, "gather idx must fit int16"
        self.debug = debug
        self.mm_bf16 = mm_bf16             # bf16 matmul operands (FWL)
        self.cnt_reg = cnt_reg             # runtime valid-count per gather
        self.mock_cc = False               # timing-sim only: no collectives


def _schedule(cfg, counts_list):
    """counts_list: per-core [TILES*NQ] edge counts. Returns per-(t,q) chunk
    counts (same for all cores = SPMD), forcing >=1 chunk per tile."""
    mx = np.max(np.stack(counts_list, 0), axis=0).reshape(cfg.TILES, cfg.NQ)
    kq = -(-mx // 128)                     # ceil
    for t in range(cfg.TILES):
        if kq[t].sum() == 0:
            kq[t][0] = 1                   # keep the program uniform
    return kq.astype(np.int64)


def _preprocess_graph(cfg, rows, cols, vals):
    """Returns (sched_kq [TILES,NQ], per-core dict of idx/val/dst arrays)."""
    rows = np.asarray(rows); cols = np.asarray(cols); vals = np.asarray(vals)
    core = rows // cfg.NPC
    per_core = []
    for c in range(cfg.CORES):
        sel = core == c
        r = rows[sel] - c * cfg.NPC
        s = cols[sel]
        v = vals[sel]
        t = r // 128
        dl = (r % 128).astype(np.float32)
        q = s // cfg.QROWS
        i16 = (s % cfg.QROWS).astype(np.int16)
        key = t * cfg.NQ + q
        order = np.argsort(key, kind="stable")
        per_core.append((key[order], i16[order], v[order].astype(np.float32),
                         dl[order]))
    counts = [np.bincount(k, minlength=cfg.TILES * cfg.NQ)
              for k, _, _, _ in per_core]
    kq = _schedule(cfg, counts)

    # chunk/column bases
    kt = kq.sum(axis=1)                               # chunks per tile
    tb = np.concatenate([[0], np.cumsum(kt)])         # tile chunk base
    TC = int(tb[-1])                                  # total chunks
    qoff = np.cumsum(kq, axis=1) - kq                 # chunk offset of q in tile
    # idx columns: per (t,q) block of kq*8 int16 columns
    ib = np.concatenate([[0], np.cumsum(kq.reshape(-1) * 8)])
    IC = int(ib[-1])

    # gather-call enumeration (t-major, q) over kq>0 cells
    call_of = -np.ones(cfg.TILES * cfg.NQ, np.int64)
    ncalls = 0
    for t in range(cfg.TILES):
        for q in range(cfg.NQ):
            if kq[t, q] > 0:
                call_of[t * cfg.NQ + q] = ncalls
                ncalls += 1

    core_arrays = []
    for (key, i16, v, dl), cnts in zip(per_core, counts):
        val_dev = np.zeros((128, TC), np.float32)
        dst_dev = np.zeros((128, TC), np.float32)
        fill = -1 if cfg.cnt_reg else 0
        idx_dev = np.full((128, IC), fill, np.int16)
        if len(key):
            gstart = np.concatenate([[0], np.cumsum(cnts)])[:-1]
            j = np.arange(len(key)) - gstart[key]     # pos within (t,q) group
            tt = key // cfg.NQ
            qq = key % cfg.NQ
            gchunk = tb[tt] + qoff[tt, qq] + j // 128
            lane = j % 128
            val_dev[lane, gchunk] = v
            dst_dev[lane, gchunk] = dl
            col = ib[key] + j // 16
            part = (j % 16).astype(np.int64)
            for g in range(8):
                idx_dev[part + 16 * g, col] = i16
        cnt_dev = np.zeros((1, max(ncalls, 1)), np.int32)
        if cfg.cnt_reg:
            for cell in range(cfg.TILES * cfg.NQ):
                ci = call_of[cell]
                if ci < 0:
                    continue
                n = int(cnts[cell])
                if n == 0:  # keep >=1 valid descriptor (idx 0, val 0)
                    idx_dev[::16, ib[cell]] = 0
                    n = 1
                cnt_dev[0, ci] = n
        core_arrays.append({"idx": idx_dev, "val": val_dev, "dst": dst_dev,
                            "cnt": cnt_dev})
    meta = {"kq": kq, "kt": kt, "tb": tb, "TC": TC, "qoff": qoff,
            "ib": ib.reshape(-1), "IC": IC, "call_of": call_of,
            "ncalls": max(ncalls, 1)}
    return meta, core_arrays


def build_program(cfg, meta_s, meta_t):
    nc = bacc.Bacc("TRN2", target_bir_lowering=False, debug=cfg.debug,
                   num_devices=cfg.CORES)
    N, D, HOP, TILES, TAIL = cfg.N, cfg.D, cfg.HOP, cfg.TILES, cfg.TAIL
    NPC, NQ, QROWS = cfg.NPC, cfg.NQ, cfg.QROWS
    graphs = ("s", "t")
    metas = {"s": meta_s, "t": meta_t}

    # ---- I/O ----
    xfull = {g: nc.dram_tensor(f"xfull_{g}", [N, D], F32, kind="ExternalInput")
             for g in graphs}
    xown = {g: nc.dram_tensor(f"xown_{g}", [TILES * 128, D], F32,
                              kind="ExternalInput") for g in graphs}
    idx_d = {g: nc.dram_tensor(f"idx_{g}", [128, metas[g]["IC"]], I16,
                               kind="ExternalInput") for g in graphs}
    val_d = {g: nc.dram_tensor(f"val_{g}", [128, metas[g]["TC"]], F32,
                               kind="ExternalInput") for g in graphs}
    dst_d = {g: nc.dram_tensor(f"dst_{g}", [128, metas[g]["TC"]], F32,
                               kind="ExternalInput") for g in graphs}
    wb_d = {g: nc.dram_tensor(f"wb_{g}", [128, HOP + 1], F32,
                              kind="ExternalInput") for g in graphs}
    cnt_d = {g: nc.dram_tensor(f"cnt_{g}", [1, metas[g]["ncalls"]], I32,
                               kind="ExternalInput") for g in graphs} \
        if cfg.cnt_reg else None
    iota_d = nc.dram_tensor("iotaf", [128, 128], F32, kind="ExternalInput")
    out_d = nc.dram_tensor("out", [NPC, 2 * D], F32, kind="ExternalOutput")

    # ---- internal DRAM for inter-hop exchange ----
    cur_nxt = {g: {h: nc.dram_tensor(f"curnxt_{g}{h}", [NPC, D], F32)
                   for h in range(1, HOP)} for g in graphs}
    cur_ful = {g: {h: nc.dram_tensor(f"curful_{g}{h}", [N, D], F32,
                                     addr_space="Shared")
                   for h in range(1, HOP)} for g in graphs}

    ktmax = max(int(metas[g]["kt"].max()) for g in graphs)

    with tile.TileContext(nc) as tc, ExitStack() as ctx:
        meta_p = ctx.enter_context(tc.tile_pool(name="meta", bufs=1))
        feat_p = ctx.enter_context(tc.tile_pool(name="feat", bufs=1))
        g_p = ctx.enter_context(tc.tile_pool(name="gather", bufs=3))
        oh_p = ctx.enter_context(tc.tile_pool(name="onehot", bufs=3))
        ps_p = ctx.enter_context(tc.tile_pool(name="psum", bufs=4,
                                              space="PSUM"))
        st_p = ctx.enter_context(tc.tile_pool(name="stage", bufs=3))

        nc.gpsimd.load_library(library_config.mlp)

        iota_f = meta_p.tile([128, 128], F32)
        nc.sync.dma_start(iota_f[:], iota_d[:, :])

        cnt_regs = None
        gt_bufs = None
        if cfg.cnt_reg:
            cnt_regs = [ctx.enter_context(nc.gpsimd.register(f"cntreg{i}"))
                        for i in range(4)]
            # Fixed gather buffers (manual round-robin): skipped (padded)
            # gather rows must read as finite so that 0*val stays 0, so we
            # zero each buffer exactly once up front.
            gt_bufs = [meta_p.tile([128, ktmax, D], F32, name=f"gtbuf{i}")
                       for i in range(3)]
            for b in gt_bufs:
                nc.vector.memset(b[:], 0.0)

        idx_t, val_t, dst_t, wb_t, feat, cnt_t = {}, {}, {}, {}, {}, {}
        for g in graphs:
            idx_t[g] = meta_p.tile([128, metas[g]["IC"]], I16, tag=f"idx{g}", name=f"idx_t_{g}")
            nc.sync.dma_start(idx_t[g][:], idx_d[g][:, :])
            val_t[g] = meta_p.tile([128, metas[g]["TC"]], F32, tag=f"val{g}", name=f"val_t_{g}")
            nc.sync.dma_start(val_t[g][:], val_d[g][:, :])
            dst_t[g] = meta_p.tile([128, metas[g]["TC"]], F32, tag=f"dst{g}", name=f"dst_t_{g}")
            nc.sync.dma_start(dst_t[g][:], dst_d[g][:, :])
            wb_t[g] = meta_p.tile([128, HOP + 1], F32, tag=f"wb{g}", name=f"wb_t_{g}")
            nc.sync.dma_start(wb_t[g][:], wb_d[g][:, :])
            if cfg.cnt_reg:
                cnt_t[g] = meta_p.tile([1, metas[g]["ncalls"]], I32,
                                       tag=f"cnt{g}", name=f"cnt_t_{g}")
                nc.sync.dma_start(cnt_t[g][:], cnt_d[g][:, :])
            # feat init: feat = w[0] * x_own
            feat[g] = feat_p.tile([128, TILES, D], F32, tag=f"feat{g}", name=f"feat_{g}")
            nc.sync.dma_start(
                feat[g][:],
                xown[g].ap().rearrange("(t p) d -> p t d", p=128))
            nc.vector.tensor_scalar_mul(
                feat[g][:].rearrange("p t d -> p (t d)"),
                feat[g][:].rearrange("p t d -> p (t d)"),
                wb_t[g][:, 0:1])

        tile_rr = 0
        for h in range(1, HOP + 1):
            for g in graphs:
                m = metas[g]
                src = xfull[g] if h == 1 else cur_ful[g][h - 1]
                for t in range(TILES):
                    kt = int(m["kt"][t])
                    if cfg.cnt_reg:
                        gt = gt_bufs[tile_rr % 3][:, :kt, :]
                        tile_rr += 1
                    else:
                        gt = g_p.tile([128, kt, D], F32, tag="gt")
                    for q in range(NQ):
                        kq = int(m["kq"][t, q])
                        if kq == 0:
                            continue
                        qo = int(m["qoff"][t, q])
                        ibase = int(m["ib"][t * NQ + q])
                        if cfg.cnt_reg:
                            ci = int(m["call_of"][t * NQ + q])
                            reg = cnt_regs[ci % 4]
                            nc.gpsimd.reg_load(reg, cnt_t[g][0:1, ci:ci + 1])
                            nreg = reg
                        else:
                            nreg = kq * 128
                        nc.gpsimd.dma_gather(
                            gt[:, qo:qo + kq, :],
                            src[q * QROWS:(q + 1) * QROWS, :],
                            idx_t[g][:, ibase:ibase + kq * 8],
                            kq * 128, nreg, D)
                    tb = int(m["tb"][t])
                    mmdt = BF16 if cfg.mm_bf16 else F32
                    oh = oh_p.tile([128, kt, 128], mmdt, tag="oh")
                    nc.vector.tensor_tensor(
                        oh[:],
                        iota_f[:].unsqueeze(1).broadcast_to([128, kt, 128]),
                        dst_t[g][:, tb:tb + kt].unsqueeze(2)
                            .broadcast_to([128, kt, 128]),
                        mybir.AluOpType.is_equal)
                    if cfg.mm_bf16:
                        rhs = oh_p.tile([128, kt, D], BF16, tag="gtb",
                                        name="gtb")
                    else:
                        rhs = gt
                    nc.vector.tensor_tensor(
                        rhs[:],
                        gt[:],
                        val_t[g][:, tb:tb + kt].unsqueeze(2)
                            .broadcast_to([128, kt, D]),
                        mybir.AluOpType.mult)
                    ps = ps_p.tile([128, D], F32)
                    for c in range(kt):
                        nc.tensor.matmul(ps[:], oh[:, c, :], rhs[:, c, :],
                                         start=(c == 0), stop=(c == kt - 1))
                    nc.vector.scalar_tensor_tensor(
                        feat[g][:, t, :], ps[:], wb_t[g][:, h:h + 1],
                        feat[g][:, t, :],
                        mybir.AluOpType.mult, mybir.AluOpType.add)
                    if h < HOP:
                        st = st_p.tile([128, D], F32)
                        nc.scalar.copy(st[:], ps[:])
                        rows = TAIL if t == TILES - 1 else 128
                        nc.sync.dma_start(
                            cur_nxt[g][h][t * 128:t * 128 + rows, :],
                            st[:rows, :])
                if h < HOP:
                    if cfg.mock_cc:
                        # timing-model stand-in for the AllGather: move the
                        # same number of received bytes through the DMA path
                        for r in range(cfg.CORES):
                            nc.sync.dma_start(
                                cur_ful[g][h][r * NPC:(r + 1) * NPC, :],
                                cur_nxt[g][h][:, :])
                    else:
                        nc.gpsimd.collective_compute(
                            "AllGather", mybir.AluOpType.bypass,
                            replica_groups=[list(range(cfg.CORES))],
                            ins=[cur_nxt[g][h].ap().opt()],
                            outs=[cur_ful[g][h].ap().opt()])

        # ---- write output: out[:, 0:D] = feat_s, out[:, D:2D] = feat_t ----
        for g, co in (("s", 0), ("t", D)):
            full_t = TILES - 1
            if full_t > 0:
                nc.sync.dma_start(
                    out_d[0:full_t * 128, co:co + D].rearrange(
                        "(t p) d -> p t d", p=128),
                    feat[g][:, 0:full_t, :])
            nc.sync.dma_start(
                out_d[full_t * 128:NPC, co:co + D],
                feat[g][0:TAIL, full_t, :])

    return nc


def _make_in_maps(cfg, inputs, meta_s, arrs_s, meta_t, arrs_t):
    x_s = np.asarray(inputs["x_s"], np.float32)
    x_t = np.asarray(inputs["x_t"], np.float32)
    w_s = np.asarray(inputs["w_s"], np.float32)
    w_t = np.asarray(inputs["w_t"], np.float32)
    iotaf = np.tile(np.arange(128, dtype=np.float32), (128, 1))
    wb_s = np.tile(w_s.reshape(1, -1), (128, 1)).astype(np.float32)
    wb_t = np.tile(w_t.reshape(1, -1), (128, 1)).astype(np.float32)
    in_maps = []
    for c in range(cfg.CORES):
        xo_s = np.zeros((cfg.TILES * 128, cfg.D), np.float32)
        xo_s[:cfg.NPC] = x_s[c * cfg.NPC:(c + 1) * cfg.NPC]
        xo_t = np.zeros((cfg.TILES * 128, cfg.D), np.float32)
        xo_t[:cfg.NPC] = x_t[c * cfg.NPC:(c + 1) * cfg.NPC]
        in_maps.append({
            "xfull_s": x_s, "xfull_t": x_t,
            "xown_s": xo_s, "xown_t": xo_t,
            "idx_s": arrs_s[c]["idx"], "val_s": arrs_s[c]["val"],
            "dst_s": arrs_s[c]["dst"],
            "idx_t": arrs_t[c]["idx"], "val_t": arrs_t[c]["val"],
            "dst_t": arrs_t[c]["dst"],
            "wb_s": wb_s, "wb_t": wb_t,
            "iotaf": iotaf,
        })
        if cfg.cnt_reg:
            in_maps[-1]["cnt_s"] = arrs_s[c]["cnt"]
            in_maps[-1]["cnt_t"] = arrs_t[c]["cnt"]
    return in_maps


def prepare(cfg, inputs):
    meta_s, arrs_s = _preprocess_graph(
        cfg, inputs["A_rows"], inputs["A_cols"], inputs["A_vals"])
    meta_t, arrs_t = _preprocess_graph(
        cfg, inputs["At_rows"], inputs["At_cols"], inputs["At_vals"])
    nc = build_program(cfg, meta_s, meta_t)
    nc.compile()
    in_maps = _make_in_maps(cfg, inputs, meta_s, arrs_s, meta_t, arrs_t)
    return nc, in_maps


def kernel(**inputs) -> np.ndarray:
    cfg = Cfg()
    nc, in_maps = prepare(cfg, inputs)
    res = run_bass_kernel_spmd(nc, in_maps, list(range(cfg.CORES)))
    return np.concatenate([res.results[c]["out"] for c in range(cfg.CORES)],
                          axis=0)
